# revision 18
# baseline (speedup 1.0000x reference)
"""Trainium2 Bass kernel for the integrate-and-fire "Integrator" layer.

Semantics (matches the JAX reference exactly):
  input  x  [4, 200, 64, 64, 8] f32, split into 2 independent time chunks of 100.
  Per neuron (b,h,w,c) and per chunk: V += x_t; if V > 2.0: spike at t, V = 0.
  Output: spike raster, permuted to [B, T, W, C, H] = [4, 200, 64, 8, 64] f32.

v6 — minimize the post-input-stream critical path under the MEASURED
hardware model (every line below was benchmarked on this device):
  - HBM ~358 GB/s per core; input 13.1 MB f32 is irreducible (the data
    is exact multiples of 2^-23, but a 3-byte split needs a 2-tensor add
    that only the DVE can do fast, and the DVE has no slack; lossy 16-bit
    FAILS outright: the dataset has near-threshold margin atoms,
    P(overshoot<1e-5)=0.27%/spike, measured rel_err 0.17).
  - DVE: 2-tensor-input ops (chain custom, SPIKE_PAIR, stt, tt) run at
    ~286 ns/row ([128,256] row); 1-tensor tensor_scalar 153; strided
    input kills the fast mode (305). PSUM residency changes nothing.
  - Scalar activation ~272 ns/row; switching activation FUNCTIONS costs
    a 1.5 us ACT_TABLE_LOAD per switch (sigmoid-only here).
  - gpsimd ("Pool") is an 8-DSP engine, ~4 G elem/s, and contends with
    the DVE for a shared SBUF port — unusable for bulk work; its
    software-DGE DMAs cost ~15 us/instruction — unusable.
  - Engines start ~7.2 us into the kernel (boot/fetch), teardown ~2.5 us.

Design: the chain (28.6 us DVE) is input-paced (36.6 us stream). Output
is 2-bit pair-coded where the DVE can afford it and raw sigma planes
where it cannot:
  - pairs of the LAST groups (steps 46..99) pack on the DVE via the
    SPIKE_PAIR custom op: p = (se<0) + 2*(so<0) in {0..3} u8, one op per
    pair row, interleaved into the input-pacing gaps between chain groups;
  - pairs of the FIRST groups (steps 0..45) ship as raw sigma planes
    {0,1} u8 computed by the otherwise-idle scalar engine (2 rows/pair);
  - all output DMAs issue on Sync after every input DMA (outputs stealing
    input bandwidth was the baseline's measured bimodality), ordered so
    early-ready rows drain first.
"""

import numpy as np

from concourse import bacc, bass, mybir
from concourse import dve_ops as _dve_ops
from concourse.dve_spec import C0, C1, Spec, Src0, Src1, Zero, _has_src1, lower, relu
from concourse.dve_uop import DveOpSpec
from concourse.tile import TileContext
from concourse.bass_utils import run_bass_kernel_spmd

_THETA = 2.0
_KBIG = 8.0   # spike marker subtracted from W; any K > theta + 1 works
_T = 100  # chunk length (time steps per independent sequence)
_P = 128  # SBUF partitions
_F = 256  # sequences per partition per core (128*256 = 32768 per core)
_NC = 8

_GROUPS = [2, 4] + [10] * 8 + [4, 4, 2, 2, 2]
_KMAX = max(_GROUPS)
_EXTRACT_DELAY = 2
_UNPACK_GS = 6   # groups 0..5 (pairs 0..22, steps 0..45) ship sigma planes;
#                  groups 6+ (27 pair rows) pack on the DVE
_B, _TT, _H, _W, _C = 4, 200, 64, 64, 8

# pairs-per-group bookkeeping
_PAIR0 = []  # first pair index of each group
_p = 0
for _kg in _GROUPS:
    _PAIR0.append(_p)
    _p += _kg // 2
_NUNPACK = _PAIR0[_UNPACK_GS]          # unpacked pairs (raw plane rows)
_RAW0 = _T // 2                        # region B start row in s


def _if_step_ref(in0, in1, s0, s1, imm2):
    # DVE relu has max(NaN, 0) = 0 semantics; inputs here are never NaN.
    w = np.maximum(np.nan_to_num(in0.astype(np.float32), nan=0.0), 0.0) + in1.astype(
        np.float32
    )
    return (w - s1 * (w > s0).astype(np.float32)).astype(np.float32)


def _spike_pair_ref(in0, in1, s0, s1, imm2):
    return (
        (in0 < 0).astype(np.float32) + s0 * (in1 < 0).astype(np.float32)
    ).astype(np.float32)


def _register_op(name, body, ref):
    for op in _dve_ops.OPS:
        if op.name == name:
            return op
    spec = Spec(body=body, reference=ref)
    row = _dve_ops._CUSTOM_DVE_ROW_BASE + len(_dve_ops.OPS)
    assert row < 0x20
    _dve_ops._SUB_OPCODE_FOR_NAME[name] = row
    ver = "v3"  # TRN2
    uops = lower(spec, ver=ver)
    sha = DveOpSpec(name=name, opcode=row, uops=uops, rd1_en=_has_src1(spec)).sha(ver)
    op = _dve_ops.DveOp(name, spec, subdim=False, uops_sha={ver: sha})
    _dve_ops.OPS.append(op)
    _dve_ops.CUSTOM_DVE_SPECS[name] = spec
    return op


_w = relu(Src0) + Src1
_IF_STEP = _register_op("IF_STEP2_ANT", _w - C1 * (_w > C0), _if_step_ref)
_SPIKE_PAIR = _register_op(
    "SPIKE_PAIR_ANT", (Src0 < Zero) + C0 * (Src1 < Zero), _spike_pair_ref
)


def _build():
    nc = bacc.Bacc("TRN2", target_bir_lowering=False, debug=False)
    # 100 rows (not padded to 128): a power-of-two 128 KB partition stride
    # put all partitions on the same HBM banks — measured ~290 GB/s vs the
    # 358 GB/s per-core share; 102.4 KB desynchronizes the banks
    x = nc.declare_dram_parameter("x", [_P, _T, _F], mybir.dt.float32,
                                  isOutput=False)
    # output rows: [0, 50) packed pair codes (groups >= _UNPACK_GS);
    # [50, 50+2*_NUNPACK) raw sigma planes (even,odd per unpacked pair)
    s = nc.declare_dram_parameter("s", [_P, 128, _F], mybir.dt.uint8, isOutput=True)
    with TileContext(nc) as tc:
        with (
            tc.tile_pool(name="xin", bufs=6) as xpool,
            tc.tile_pool(name="sout", bufs=1) as spool,
            tc.tile_pool(name="state", bufs=1) as stpool,
        ):
            # persistent state history: row r = state after step r (row 0 = 0)
            og = stpool.tile([_P, _T + 1, _F], mybir.dt.float32, tag="og")
            nc.vector.memset(og[:, 0, :], 0.0)
            # persistent output staging (pair codes + raw planes)
            so = spool.tile([_P, _RAW0 + 2 * _NUNPACK, _F], mybir.dt.uint8, tag="s")

            def emit_extract(ge, t0e, kge):
                p0 = _PAIR0[ge]
                np_ = kge // 2
                if ge < _UNPACK_GS:
                    # raw sigma planes on the scalar engine: {0,1} u8
                    # (sigmoid saturates; exactly-0 states round to 0).
                    # Per-group block: np_ even-step rows then np_ odd rows
                    # (contiguous writes; strided writes hit the slow path)
                    r = _RAW0 + 2 * p0
                    nc.scalar.activation(
                        out=so[:, r:r + np_, :],
                        in_=og[:, t0e + 1:t0e + kge:2, :],
                        func=mybir.ActivationFunctionType.Sigmoid,
                        bias=0.0, scale=-1e30,
                    )
                    nc.scalar.activation(
                        out=so[:, r + np_:r + 2 * np_, :],
                        in_=og[:, t0e + 2:t0e + kge + 1:2, :],
                        func=mybir.ActivationFunctionType.Sigmoid,
                        bias=0.0, scale=-1e30,
                    )
                else:
                    # packed pairs on the DVE: p = (se<0) + 2*(so<0)
                    nc.vector._custom_dve(
                        _SPIKE_PAIR,
                        out=so[:, p0:p0 + np_, :],
                        in0=og[:, t0e + 1:t0e + kge:2, :],
                        in1=og[:, t0e + 2:t0e + kge + 1:2, :],
                        s0=2.0, s1=0.0, imm2=0.0,
                    )

            pending = []
            t0 = 0
            for g, kg in enumerate(_GROUPS):
                xt = xpool.tile([_P, _KMAX, _F], mybir.dt.float32, tag="x")
                nc.sync.dma_start(out=xt[:, :kg, :], in_=x[:, t0:t0 + kg, :])
                # whole group's recurrence in ONE instruction (the state
                # feeds through SBUF within the instruction)
                nc.vector._custom_dve(
                    _IF_STEP,
                    out=og[:, t0 + 1:t0 + 1 + kg, :],
                    in0=og[:, t0:t0 + kg, :],
                    in1=xt[:, :kg, :],
                    s0=_THETA,
                    s1=_KBIG,
                    imm2=0.0,
                )
                pending.append((g, t0, kg))
                if len(pending) > _EXTRACT_DELAY:
                    emit_extract(*pending.pop(0))
                t0 += kg
            for p in pending:
                emit_extract(*p)
            # output DMAs strictly after every input DMA on the Sync queue;
            # early-ready regions first so rings drain while the tail computes
            pk0 = _PAIR0[_UNPACK_GS]
            chunks = [
                (_RAW0, _RAW0 + 2 * _NUNPACK),   # raw planes (ready early)
                (pk0, (pk0 + _T // 2) // 2),     # first packed half
                ((pk0 + _T // 2) // 2, _T // 2),  # tail packed rows
            ]
            for r0, r1 in chunks:
                nc.sync.dma_start(out=s[:, r0:r1, :], in_=so[:, r0:r1, :])
    return nc


def _shard(x):
    # [B, 200, H, W, C] -> per-core [128, 100->128, 256] f32, sequence-major
    xr = (
        x.reshape(_B, 2, _T, _H, _W, _C)
        .transpose(0, 1, 3, 4, 5, 2)  # [b, chunk, h, w, c, t]
        .reshape(-1, _T)              # [262144, 100]
    )
    per_core = xr.reshape(_NC, _P, _F, _T).transpose(0, 1, 3, 2)  # [8,128,100,256]
    return [{"x": np.ascontiguousarray(per_core[c])} for c in range(_NC)]


def _unshard(core_outs):
    # [128, 128, 256] u8: rows [0,50) packed codes, [50,50+2u) raw planes
    raw = np.stack([np.asarray(o) for o in core_outs])
    full = np.zeros((_NC, _P, _T, _F), np.float32)
    # unpacked pairs: per-group blocks of [evens | odds] sigma rows
    for ge in range(_UNPACK_GS):
        p0, np_ = _PAIR0[ge], _GROUPS[ge] // 2
        r = _RAW0 + 2 * p0
        t0e = 2 * p0
        full[:, :, t0e:t0e + 2 * np_:2, :] = raw[:, :, r:r + np_, :]
        full[:, :, t0e + 1:t0e + 2 * np_ + 1:2, :] = raw[:, :, r + np_:r + 2 * np_, :]
    u = _NUNPACK
    # packed pairs: p = even + 2*odd
    pk = raw[:, :, _PAIR0[_UNPACK_GS]:_T // 2, :]
    full[:, :, 2 * u:_T:2, :] = (pk & 1).astype(np.float32)
    full[:, :, 2 * u + 1:_T:2, :] = (pk >> 1).astype(np.float32)
    sp = full.transpose(0, 1, 3, 2).reshape(_B, 2, _H, _W, _C, _T)  # [b,k,h,w,c,t]
    out = sp.transpose(0, 1, 5, 3, 4, 2).reshape(_B, _TT, _W, _C, _H)
    return np.ascontiguousarray(out)


def _run(x, trace=False):
    nc = _build()
    nc.finalize()  # run Bacc passes (multi-wait splitting etc.); PJRT path skips it
    in_maps = _shard(np.asarray(x, dtype=np.float32))
    res = run_bass_kernel_spmd(nc, in_maps, core_ids=list(range(_NC)), trace=trace)
    out = _unshard([r["s"] for r in res.results])
    return out, res


def kernel(inputs):
    out, _ = _run(inputs, trace=False)
    return out
